# revision 1
# baseline (speedup 1.0000x reference)
"""Trainium2 Bass kernel for a causal multi-head attention layer.

Model: b=2, s=2048, d_model=1024, 16 heads, head_dim=64, pad-index 0.
Sharding over 8 NeuronCores: each core owns 2 heads (128 of the 1024
attention dims) for both batches (head/tensor parallel).  After attention,
an AllToAll redistributes the per-head outputs so each core holds all 1024
attention dims for 1/8 of the sequence positions, where it runs the output
projection locally.  Output rows per core: 256 rows of each batch.
"""

import threading

import numpy as np

B, S, D = 2, 2048, 1024
H, HD = 16, 64
NCORES = 8
LD = D // NCORES          # 128 local attention dims (2 heads)
R = B * S                 # 4096 flattened rows
RC = R // NCORES          # 512 output rows per core
RB = S // NCORES          # 256 rows per batch per core
NKT = S // 128            # 16 key tiles per batch
NCH = D // 128            # 8 contraction chunks of d_model

_cache = {}
_lock = threading.Lock()


def _build_nc():
    import concourse.mybir as mybir
    import concourse.tile as tile
    from concourse import bacc
    from concourse.masks import make_identity
    from contextlib import ExitStack

    f32 = mybir.dt.float32
    bf16 = mybir.dt.bfloat16
    i32 = mybir.dt.int32
    AF = mybir.ActivationFunctionType
    ALU = mybir.AluOpType

    nc = bacc.Bacc(None, target_bir_lowering=False, num_devices=NCORES)

    xT = nc.declare_dram_parameter("xT", [D, R], bf16, isOutput=False)
    wqT = nc.declare_dram_parameter("wqT", [D, LD], bf16, isOutput=False)
    wkT = nc.declare_dram_parameter("wkT", [D, LD], bf16, isOutput=False)
    wvT = nc.declare_dram_parameter("wvT", [D, LD], bf16, isOutput=False)
    woT = nc.declare_dram_parameter("woT", [D, D], bf16, isOutput=False)
    bq = nc.declare_dram_parameter("bq", [LD], f32, isOutput=False)
    bk = nc.declare_dram_parameter("bk", [LD], f32, isOutput=False)
    bv = nc.declare_dram_parameter("bv", [LD], f32, isOutput=False)
    bo = nc.declare_dram_parameter("bo", [D], f32, isOutput=False)
    ids = nc.declare_dram_parameter("ids", [128, B * NKT], i32, isOutput=False)
    out = nc.declare_dram_parameter("out", [RC, D], f32, isOutput=True)

    with ExitStack() as ctx:
        tc = ctx.enter_context(tile.TileContext(nc))
        const = ctx.enter_context(tc.tile_pool(name="const", bufs=1))
        qkp = ctx.enter_context(tc.tile_pool(name="qkp", bufs=2))
        work = ctx.enter_context(tc.tile_pool(name="work", bufs=4))
        est = ctx.enter_context(tc.tile_pool(name="est", bufs=1))
        stg = ctx.enter_context(tc.tile_pool(name="stg", bufs=2))
        spool = ctx.enter_context(tc.tile_pool(name="spool", bufs=2, space="PSUM"))
        opool = ctx.enter_context(tc.tile_pool(name="opool", bufs=4, space="PSUM"))
        dpool = ctx.enter_context(tc.tile_pool(name="dram", bufs=2, space="DRAM"))

        # ---- constants (small weights first so compute can start early) ----
        wqT_sb = const.tile([128, NCH, LD], bf16)
        nc.sync.dma_start(wqT_sb, wqT.ap().rearrange("(c p) d -> p c d", p=128))
        wkT_sb = const.tile([128, NCH, LD], bf16)
        nc.sync.dma_start(wkT_sb, wkT.ap().rearrange("(c p) d -> p c d", p=128))
        wvT_sb = const.tile([128, NCH, LD], bf16)
        nc.sync.dma_start(wvT_sb, wvT.ap().rearrange("(c p) d -> p c d", p=128))
        # x^T loaded as 8 independent contraction-chunk tiles so projection
        # matmuls on chunk c start as soon as chunk c lands
        xTr = xT.ap().rearrange("(c p) r -> c p r", p=128)
        xT_ch = []
        for c in range(NCH):
            xc = const.tile([128, R], bf16, name=f"xc{c}", tag=f"xc{c}")
            nc.sync.dma_start(xc, xTr[c])
            xT_ch.append(xc)
        woT_sb = const.tile([128, NCH, D], bf16)
        nc.sync.dma_start(woT_sb, woT.ap().rearrange("(c p) n -> p c n", p=128))

        bq_col = const.tile([128, 1], f32)
        nc.sync.dma_start(bq_col, bq.ap().rearrange("(p o) -> p o", o=1))
        bk_col = const.tile([128, 1], f32)
        nc.sync.dma_start(bk_col, bk.ap().rearrange("(p o) -> p o", o=1))
        bv_bc = const.tile([128, LD], f32)
        nc.sync.dma_start(bv_bc, bv.ap().partition_broadcast(128))
        bo_bc = const.tile([128, D], f32)
        nc.sync.dma_start(bo_bc, bo.ap().partition_broadcast(128))

        ids_sb = const.tile([128, B * NKT], i32)
        nc.sync.dma_start(ids_sb, ids.ap())
        padf = const.tile([128, B * NKT], f32)
        nc.vector.tensor_copy(padf, ids_sb)
        nc.vector.tensor_scalar_min(padf, padf, 1.0)

        ident = const.tile([128, 128], bf16)
        make_identity(nc, ident)
        # diagmask[x, y] = 1 if y >= x else 0  (keys on partitions, queries on free)
        diagmask = const.tile([128, 128], bf16)
        nc.gpsimd.memset(diagmask, 1.0)
        nc.gpsimd.affine_select(
            out=diagmask, in_=diagmask, compare_op=ALU.is_ge, fill=0.0,
            base=0, pattern=[[1, 128]], channel_multiplier=-1,
        )

        a2a_outs = []
        for b in range(B):
            # ---- projections for batch b ----
            # QT/KT: [128 dims(2 heads), 2048 rows]; v_aug: [128 keys, head, kt, 65]
            qt_sb = qkp.tile([128, S], bf16, name=f"qt{b}", tag="qt")
            kt_sb = qkp.tile([128, S], bf16, name=f"kt{b}", tag="kt")
            vaug = qkp.tile([128, 2, NKT, HD + 1], bf16, name=f"vaug{b}", tag="vaug")
            # Q/K computed directly in [dims, rows] layout (both heads: M=128)
            for ch in range(S // 512):
                rsl = slice(b * S + ch * 512, b * S + (ch + 1) * 512)
                csl = slice(ch * 512, (ch + 1) * 512)
                pqt = opool.tile([128, 512], f32, name="pqt", tag="o")
                pkt = opool.tile([128, 512], f32, name="pkt", tag="o")
                for c in range(NCH):
                    st = c == 0
                    sp = c == NCH - 1
                    rhs = xT_ch[c][:, rsl]
                    nc.tensor.matmul(pqt, wqT_sb[:, c, :], rhs, start=st, stop=sp)
                    nc.tensor.matmul(pkt, wkT_sb[:, c, :], rhs, start=st, stop=sp)
                nc.vector.tensor_scalar_add(qt_sb[:, csl], pqt, bq_col)
                nc.vector.tensor_scalar_add(kt_sb[:, csl], pkt, bk_col)
            # V in [keys, dims] layout for the PV matmul
            for m in range(NKT):
                rsl = slice(b * S + m * 128, b * S + (m + 1) * 128)
                pv = opool.tile([128, LD], f32, name="pv", tag="o")
                for c in range(NCH):
                    nc.tensor.matmul(pv, xT_ch[c][:, rsl], wvT_sb[:, c, :],
                                     start=(c == 0), stop=(c == NCH - 1))
                # bias, pad-zero rows, ones column (also pad-zeroed)
                tv = work.tile([128, LD], f32, name="tv", tag="tv")
                nc.vector.tensor_add(tv, pv, bv_bc)
                pcol = padf[:, b * NKT + m:b * NKT + m + 1]
                for h in range(2):
                    nc.vector.tensor_scalar_mul(
                        vaug[:, h, m, 0:HD], tv[:, h * HD:(h + 1) * HD], pcol)
                    nc.vector.tensor_copy(vaug[:, h, m, HD:HD + 1], pcol)

            # ---- attention for batch b, heads h=0,1 (local) ----
            stage = stg.tile([128, S], bf16, name=f"stage{b}", tag="stage")
            for h in range(2):
                hsl = slice(h * HD, (h + 1) * HD)
                ests = []

                def do_pv(m, h=h, hsl=hsl, vaug=vaug, stage=stage, ests=ests):
                    po = opool.tile([128, HD + 1], f32, name="po", tag="o")
                    for k2 in range(m + 1):
                        nc.tensor.matmul(
                            po,
                            ests[k2][:, (m - k2) * 128:(m - k2) * 128 + 128],
                            vaug[:, h, k2, :],
                            start=(k2 == 0), stop=(k2 == m))
                    rec = work.tile([128, 1], f32, name="rec", tag="rec")
                    nc.vector.reciprocal(rec, po[:, HD:HD + 1])
                    at = work.tile([128, HD], bf16, name="at", tag="at")
                    nc.vector.tensor_scalar_mul(at, po[:, 0:HD], rec)
                    pt = spool.tile([128, 128], bf16, name="pt", tag="s")
                    nc.tensor.transpose(pt[0:HD, :], at, ident)
                    nc.vector.tensor_copy(
                        stage[hsl, m * 128:(m + 1) * 128], pt[0:HD, :])

                for kt in range(NKT):
                    q0 = kt * 128          # first visible query
                    w = S - q0             # width of this kt row
                    e = est.tile([128, w], bf16, name=f"e{kt}", tag=f"e{kt}")
                    ests.append(e)
                    # scores in <=1024-wide chunks, exp each chunk
                    off = 0
                    while off < w:
                        cw = min(1024, w - off)
                        ps = spool.tile([128, 1024], f32, name="ps", tag="s")
                        o2 = 0
                        while o2 < cw:
                            mw = min(512, cw - o2)
                            nc.tensor.matmul(
                                ps[:, o2:o2 + mw],
                                kt_sb[hsl, kt * 128:(kt + 1) * 128],
                                qt_sb[hsl, q0 + off + o2:q0 + off + o2 + mw],
                                start=True, stop=True)
                            o2 += mw
                        nc.scalar.activation(
                            e[:, off:off + cw], ps[:, 0:cw], AF.Exp, scale=0.125)
                        off += cw
                    # causal mask on the diagonal 128 columns
                    nc.vector.tensor_mul(e[:, 0:128], e[:, 0:128], diagmask)
                    # PV shifted one kt behind scores so the tensor engine is
                    # never waiting on the exp it just requested
                    if kt >= 1:
                        do_pv(kt - 1)
                do_pv(NKT - 1)

            # ---- AllToAll for batch b, two q-half chunks ----
            # chunk t covers batch rows [t*1024, (t+1)*1024); each core ends
            # up with rows [t*1024 + core*128, +128) of this batch
            for t in range(2):
                a2a_in = dpool.tile([NCORES * 128, 128], bf16,
                                    name=f"a2ai{b}{t}", tag="a2ai", bufs=4)
                nc.sync.dma_start(
                    a2a_in.rearrange("(j p) r -> p j r", p=128),
                    stage[:, t * 1024:(t + 1) * 1024]
                    .rearrange("p (j r) -> p j r", j=NCORES))
                a2a_out = dpool.tile([NCORES * 128, 128], bf16,
                                     name=f"a2ao{b}{t}", tag="a2ao", bufs=4)
                nc.gpsimd.collective_compute(
                    "AllToAll", ALU.bypass,
                    replica_groups=[list(range(NCORES))],
                    ins=[a2a_in.opt()], outs=[a2a_out.opt()])
                a2a_outs.append((b, t, a2a_out))

        # ---- output projection (128-row chunks; b0 overlaps b1's A2As) ----
        for b, t, a2a_out in a2a_outs:
            a2a_sb = stg.tile([128, NCORES, 128], bf16, name=f"a2as{b}{t}",
                              tag="a2as", bufs=4)
            nc.sync.dma_start(
                a2a_sb, a2a_out.rearrange("(j p) r -> p j r", p=128))
            r0 = b * RB + t * 128
            for n in range(D // 512):
                pout = opool.tile([128, 512], f32, name="pout", tag="o")
                for c in range(NCH):
                    nc.tensor.matmul(
                        pout,
                        a2a_sb[:, c, :],
                        woT_sb[:, c, n * 512:(n + 1) * 512],
                        start=(c == 0), stop=(c == NCH - 1))
                ot = work.tile([128, 512], f32, name="ot", tag="ot")
                nc.vector.tensor_add(ot, pout, bo_bc[:, n * 512:(n + 1) * 512])
                nc.sync.dma_start(
                    out.ap()[r0:r0 + 128, n * 512:(n + 1) * 512], ot)

    nc.finalize()
    return nc


def _get_nc():
    with _lock:
        if "nc" not in _cache:
            _cache["nc"] = _build_nc()
        return _cache["nc"]


def _shard_inputs(x, input_ids, Wq, bq, Wk, bk, Wv, bv, Wo, bo):
    import ml_dtypes
    bf16 = ml_dtypes.bfloat16

    x = np.asarray(x, dtype=np.float32)
    xT = np.ascontiguousarray(x.reshape(R, D).T).astype(bf16)
    woT = np.ascontiguousarray(np.asarray(Wo, dtype=np.float32).T).astype(bf16)
    bo_f = np.asarray(bo, dtype=np.float32)
    ids = np.asarray(input_ids).astype(np.int32)
    # ids_r[p, b*NKT + t] = input_ids[b, t*128 + p]
    ids_r = np.ascontiguousarray(ids.reshape(B, NKT, 128).transpose(2, 0, 1)
                                 .reshape(128, B * NKT))
    Wq = np.asarray(Wq, dtype=np.float32)
    Wk = np.asarray(Wk, dtype=np.float32)
    Wv = np.asarray(Wv, dtype=np.float32)
    bq = np.asarray(bq, dtype=np.float32)
    bk = np.asarray(bk, dtype=np.float32)
    bv = np.asarray(bv, dtype=np.float32)

    in_maps = []
    for c in range(NCORES):
        sl = slice(c * LD, (c + 1) * LD)
        in_maps.append({
            "xT": xT,
            "wqT": np.ascontiguousarray(Wq[sl].T).astype(bf16),
            "wkT": np.ascontiguousarray(Wk[sl].T).astype(bf16),
            "wvT": np.ascontiguousarray(Wv[sl].T).astype(bf16),
            "woT": woT,
            "bq": bq[sl].copy(),
            "bk": bk[sl].copy(),
            "bv": bv[sl].copy(),
            "bo": bo_f,
            "ids": ids_r,
        })
    return in_maps


def run(trace=False, **inputs):
    """Run the kernel; returns (output, BassKernelResults)."""
    from concourse.bass_utils import run_bass_kernel_spmd

    nc = _get_nc()
    in_maps = _shard_inputs(**inputs)
    res = run_bass_kernel_spmd(nc, in_maps, core_ids=list(range(NCORES)),
                               trace=trace)
    full = np.empty((B, S, D), dtype=np.float32)
    for c in range(NCORES):
        o = np.asarray(res.results[c]["out"], dtype=np.float32)
        for b in range(B):
            for t in range(2):
                full[b, t * 1024 + c * 128:t * 1024 + (c + 1) * 128, :] = \
                    o[b * RB + t * 128:b * RB + (t + 1) * 128, :]
    return full, res


def kernel(**inputs) -> np.ndarray:
    full, _ = run(trace=False, **inputs)
    return full



# revision 5
# speedup vs baseline: 1.0192x; 1.0192x over previous
"""Trainium2 Bass kernel for a causal multi-head attention layer.

Model: b=2, s=2048, d_model=1024, 16 heads, head_dim=64, pad-index 0.
Sharding over 8 NeuronCores: each core owns 2 heads (128 of the 1024
attention dims) for both batches (head/tensor parallel).  After attention,
an AllToAll redistributes the per-head outputs so each core holds all 1024
attention dims for 1/8 of the sequence positions, where it runs the output
projection locally.  Output rows per core: 256 rows of each batch.

v2 layout/schedule:
  - scores for the two local heads run concurrently on the PE array via
    row tiling (K=64 contraction at PE rows 0-63 / 64-127).
  - PV is computed "flipped" (V-with-ones-column stationary, exp(scores)
    moving), so the output lands as [dims, queries] -- no PE transposes.
    The 65th row of the PSUM accumulator is the softmax denominator.
  - attention runs in 512-query passes (exact causal staircase);
    normalization = DVE reciprocal + K=1 broadcast matmul + DVE multiply.
  - projection matmuls for the *other* batch and the output projections
    are interleaved into the attention instruction stream as PE filler so
    the tensor engine never idles (keeps the HAM clock gate at 8/8).
  - AllToAll chunks (1024 queries) are issued as soon as their passes
    finish, overlapping the remaining attention compute.
"""

import threading

import numpy as np

B, S, D = 2, 2048, 1024
H, HD = 16, 64
NCORES = 8
LD = D // NCORES          # 128 local attention dims (2 heads)
R = B * S                 # 4096 flattened rows
RC = R // NCORES          # 512 output rows per core
RB = S // NCORES          # 256 rows per batch per core
NKT = S // 128            # 16 key tiles per batch
NCH = D // 128            # 8 contraction chunks of d_model
PASS = 512                # attention query-pass width
NPASS = S // PASS         # 4 passes per batch

_cache = {}
_lock = threading.Lock()


class _Filler:
    """FIFO of instruction-emitting thunks, pumped between attention ops
    to keep the tensor engine busy during softmax-bound stretches."""

    def __init__(self):
        self.gens = []

    def add(self, gen):
        self.gens.append(gen)

    def pump(self, n):
        done = 0
        while done < n and self.gens:
            try:
                next(self.gens[0])()
                done += 1
            except StopIteration:
                self.gens.pop(0)

    def drain(self):
        while self.gens:
            self.pump(1)


def _build_nc():
    import concourse.mybir as mybir
    import concourse.tile as tile
    from concourse import bacc
    from contextlib import ExitStack

    f32 = mybir.dt.float32
    bf16 = mybir.dt.bfloat16
    i32 = mybir.dt.int32
    AF = mybir.ActivationFunctionType
    ALU = mybir.AluOpType

    nc = bacc.Bacc(None, target_bir_lowering=False, num_devices=NCORES)

    xT = nc.declare_dram_parameter("xT", [D, R], bf16, isOutput=False)
    wqT = nc.declare_dram_parameter("wqT", [D, LD], bf16, isOutput=False)
    wkT = nc.declare_dram_parameter("wkT", [D, LD], bf16, isOutput=False)
    wvT = nc.declare_dram_parameter("wvT", [D, LD], bf16, isOutput=False)
    woT = nc.declare_dram_parameter("woT", [D, D], bf16, isOutput=False)
    bq = nc.declare_dram_parameter("bq", [LD], f32, isOutput=False)
    bk = nc.declare_dram_parameter("bk", [LD], f32, isOutput=False)
    bv = nc.declare_dram_parameter("bv", [LD], f32, isOutput=False)
    bo = nc.declare_dram_parameter("bo", [D], f32, isOutput=False)
    ids = nc.declare_dram_parameter("ids", [128, B * NKT], i32, isOutput=False)
    out = nc.declare_dram_parameter("out", [RC, D], f32, isOutput=True)

    with ExitStack() as ctx:
        tc = ctx.enter_context(tile.TileContext(nc))
        const = ctx.enter_context(tc.tile_pool(name="const", bufs=1))
        qkp = ctx.enter_context(tc.tile_pool(name="qkp", bufs=2))
        work = ctx.enter_context(tc.tile_pool(name="work", bufs=4))
        epool = ctx.enter_context(tc.tile_pool(name="epool", bufs=3))
        stg = ctx.enter_context(tc.tile_pool(name="stg", bufs=2))
        dpool = ctx.enter_context(tc.tile_pool(name="dram", bufs=2, space="DRAM"))

        # ---- constants (small weights first so compute can start early) ----
        wqT_sb = const.tile([128, NCH, LD], bf16)
        nc.sync.dma_start(wqT_sb, wqT.ap().rearrange("(c p) d -> p c d", p=128))
        wkT_sb = const.tile([128, NCH, LD], bf16)
        nc.sync.dma_start(wkT_sb, wkT.ap().rearrange("(c p) d -> p c d", p=128))
        wvT_sb = const.tile([128, NCH, LD], bf16)
        nc.sync.dma_start(wvT_sb, wvT.ap().rearrange("(c p) d -> p c d", p=128))
        bq_col = const.tile([128, 1], f32)
        nc.sync.dma_start(bq_col, bq.ap().rearrange("(p o) -> p o", o=1))
        bk_col = const.tile([128, 1], f32)
        nc.sync.dma_start(bk_col, bk.ap().rearrange("(p o) -> p o", o=1))
        bv_bc = const.tile([128, LD], f32)
        nc.sync.dma_start(bv_bc, bv.ap().partition_broadcast(128))
        ids_sb = const.tile([128, B * NKT], i32)
        nc.sync.dma_start(ids_sb, ids.ap())

        # x^T per (batch, contraction chunk): [128, S] tiles.  Batch 0's
        # chunks first so its QK projection starts after the first lands.
        xTr = xT.ap().rearrange("(c p) (b r) -> b c p r", p=128, b=B)
        xb = [[None] * NCH for _ in range(B)]
        for b in range(B):
            for c in range(NCH):
                t = const.tile([128, S], bf16, name=f"x{b}c{c}", tag=f"x{b}c{c}")
                nc.sync.dma_start(t, xTr[b, c])
                xb[b][c] = t
        woT_sb = const.tile([128, NCH, D], bf16)
        nc.sync.dma_start(woT_sb, woT.ap().rearrange("(c p) n -> p c n", p=128))
        bo_bc = const.tile([128, D], f32)
        nc.sync.dma_start(bo_bc, bo.ap().partition_broadcast(128))

        padf = const.tile([128, B * NKT], f32)
        nc.vector.tensor_copy(padf, ids_sb)
        nc.vector.tensor_scalar_min(padf, padf, 1.0)

        ones_row = const.tile([1, HD], bf16)
        nc.gpsimd.memset(ones_row, 1.0)
        # diagmask[x, y] = 1 if y >= x else 0  (keys on partitions)
        diagmask = const.tile([128, 128], bf16)
        nc.gpsimd.memset(diagmask, 1.0)
        nc.gpsimd.affine_select(
            out=diagmask, in_=diagmask, compare_op=ALU.is_ge, fill=0.0,
            base=0, pattern=[[1, 128]], channel_multiplier=-1,
        )

        qt = [None] * B
        kt = [None] * B
        vaug = [None] * B
        stage = [None] * B

        # ---- batch 0 QK projection, contraction-outer (DMA-pipelined) ----
        # Uses 8 PSUM banks transiently; the pool closes before the
        # attention-phase PSUM pools open.
        qt[0] = qkp.tile([128, S], bf16, name="qt0", tag="qt")
        kt[0] = qkp.tile([128, S], bf16, name="kt0", tag="kt")
        with tc.tile_pool(name="qk8", bufs=1, space="PSUM") as qk8:
            pq = qk8.tile([128, S], f32, name="pq0", tag="pq")
            pk = qk8.tile([128, S], f32, name="pk0", tag="pk")
            for c in range(NCH):
                st, sp = c == 0, c == NCH - 1
                for q4 in range(S // 512):
                    sl = slice(q4 * 512, (q4 + 1) * 512)
                    nc.tensor.matmul(pq[:, sl], wqT_sb[:, c, :],
                                     xb[0][c][:, sl], start=st, stop=sp)
                    nc.tensor.matmul(pk[:, sl], wkT_sb[:, c, :],
                                     xb[0][c][:, sl], start=st, stop=sp)
            for q4 in range(S // 512):
                sl = slice(q4 * 512, (q4 + 1) * 512)
                nc.vector.tensor_scalar_add(qt[0][:, sl], pq[:, sl], bq_col)
                nc.vector.tensor_scalar_add(kt[0][:, sl], pk[:, sl], bk_col)

        # ---- attention-phase PSUM pools (exactly 8 banks) ----
        sc0 = ctx.enter_context(tc.tile_pool(name="sc0", bufs=1, space="PSUM"))
        sc1 = ctx.enter_context(tc.tile_pool(name="sc1", bufs=1, space="PSUM"))
        pvp = ctx.enter_context(tc.tile_pool(name="pvp", bufs=1, space="PSUM"))
        fil = ctx.enter_context(tc.tile_pool(name="fil", bufs=2, space="PSUM"))

        def v_proj_thunks(b):
            """V projection in [keys, dims] layout + bias/pad/ones -> vaug."""
            vaug[b] = qkp.tile([128, 2, NKT, HD + 1], bf16,
                               name=f"vaug{b}", tag="vaug")
            for m in range(NKT):
                def go(m=m, b=b):
                    rsl = slice(m * 128, (m + 1) * 128)
                    pvt = fil.tile([128, 512], f32, name="pv", tag="fil")
                    for c in range(NCH):
                        nc.tensor.matmul(pvt[:, 0:LD], xb[b][c][:, rsl],
                                         wvT_sb[:, c, :],
                                         start=(c == 0), stop=(c == NCH - 1))
                    tv = work.tile([128, LD], f32, name="tv", tag="tv")
                    nc.vector.tensor_add(tv, pvt[:, 0:LD], bv_bc)
                    pcol = padf[:, b * NKT + m:b * NKT + m + 1]
                    for h in range(2):
                        nc.vector.tensor_scalar_mul(
                            vaug[b][:, h, m, 0:HD], tv[:, h * HD:(h + 1) * HD],
                            pcol)
                        nc.vector.tensor_copy(vaug[b][:, h, m, HD:HD + 1], pcol)
                yield go

        def qk_proj_thunks(b):
            """QK projection as filler thunks (x for batch b must be fully
            resident by the time these are pumped)."""
            qt[b] = qkp.tile([128, S], bf16, name=f"qt{b}", tag="qt")
            kt[b] = qkp.tile([128, S], bf16, name=f"kt{b}", tag="kt")
            for q4 in range(S // 512):
                def go(q4=q4, b=b):
                    sl = slice(q4 * 512, (q4 + 1) * 512)
                    pqt = fil.tile([128, 512], f32, name="pq", tag="fil")
                    for c in range(NCH):
                        nc.tensor.matmul(pqt, wqT_sb[:, c, :], xb[b][c][:, sl],
                                         start=(c == 0), stop=(c == NCH - 1))
                    nc.vector.tensor_scalar_add(qt[b][:, sl], pqt, bq_col)
                yield go

                def go2(q4=q4, b=b):
                    sl = slice(q4 * 512, (q4 + 1) * 512)
                    pkt = fil.tile([128, 512], f32, name="pk", tag="fil")
                    for c in range(NCH):
                        nc.tensor.matmul(pkt, wkT_sb[:, c, :], xb[b][c][:, sl],
                                         start=(c == 0), stop=(c == NCH - 1))
                    nc.vector.tensor_scalar_add(kt[b][:, sl], pkt, bk_col)
                yield go2

        def outproj_thunks(b, pp, a2a_out):
            """Output projection for one gathered 128-row chunk."""
            a2a_sb = stg.tile([128, NCORES, 128], bf16, name=f"a2as{b}{pp}",
                              tag="a2as", bufs=4)

            def load(a2a_sb=a2a_sb, a2a_out=a2a_out):
                nc.sync.dma_start(
                    a2a_sb, a2a_out.rearrange("(j p) r -> p j r", p=128))
            yield load
            r0 = b * RB + pp * 128
            for n in range(D // 512):
                def go(n=n, a2a_sb=a2a_sb, r0=r0):
                    pout = fil.tile([128, 512], f32, name="po", tag="fil")
                    for c in range(NCH):
                        nc.tensor.matmul(
                            pout, a2a_sb[:, c, :],
                            woT_sb[:, c, n * 512:(n + 1) * 512],
                            start=(c == 0), stop=(c == NCH - 1))
                    ot = work.tile([128, 512], f32, name="ot", tag="ot")
                    nc.vector.tensor_add(ot, pout,
                                         bo_bc[:, n * 512:(n + 1) * 512])
                    nc.sync.dma_start(
                        out.ap()[r0:r0 + 128, n * 512:(n + 1) * 512], ot)
                yield go

        filler = _Filler()

        def issue_a2a(b, pp):
            """AllToAll one 1024-query chunk of batch b's stage buffer; its
            output projection becomes filler work."""
            a2a_in = dpool.tile([NCORES * 128, 128], bf16,
                                name=f"a2ai{b}{pp}", tag="a2ai", bufs=4)
            nc.sync.dma_start(
                a2a_in.rearrange("(j p) r -> p j r", p=128),
                stage[b][:, pp * 1024:(pp + 1) * 1024]
                .rearrange("p (j r) -> p j r", j=NCORES))
            a2a_out = dpool.tile([NCORES * 128, 128], bf16,
                                 name=f"a2ao{b}{pp}", tag="a2ao", bufs=4)
            nc.gpsimd.collective_compute(
                "AllToAll", ALU.bypass,
                replica_groups=[list(range(NCORES))],
                ins=[a2a_in.opt()], outs=[a2a_out.opt()])
            filler.add(outproj_thunks(b, pp, a2a_out))

        def attention(b, pump_from=0):
            """Attention for batch b, both heads, 512-query passes.  PV is
            pipelined one score-pair behind exp."""
            stage[b] = stg.tile([128, S], bf16, name=f"stage{b}", tag="stage")
            for p in range(NPASS):
                q0 = p * PASS
                ks = list(range(4 * p + 4))      # visible key tiles
                pairs = []
                for i in range(0, len(ks), 2):
                    grp = ks[i:i + 2]
                    pairs.append([(k2, min(PASS, q0 + PASS - 128 * k2))
                                  for k2 in grp])
                npair = len(pairs)
                pv0 = pvp.tile([HD + 1, PASS], f32, name="pv0", tag="pv0")
                pv1 = pvp.tile([HD + 1, PASS], f32, name="pv1", tag="pv1")
                etiles = [None] * npair

                def do_pv(pi):
                    e0, e1, widths = etiles[pi]
                    off = 0
                    for j, (k2, w) in enumerate(widths):
                        st = pi == 0 and j == 0
                        sp = pi == npair - 1 and j == len(widths) - 1
                        psl = slice(PASS - w, PASS)
                        nc.tensor.matmul(pv0[:, psl], vaug[b][:, 0, k2, :],
                                         e0[:, off:off + w], start=st, stop=sp)
                        nc.tensor.matmul(pv1[:, psl], vaug[b][:, 1, k2, :],
                                         e1[:, off:off + w], start=st, stop=sp)
                        off += w

                for pi, widths in enumerate(pairs):
                    # scores for this pair, both heads (row-tiled, concurrent)
                    sp0 = sc0.tile([128, 1024], f32, name="s0", tag="s0")
                    sp1 = sc1.tile([128, 1024], f32, name="s1", tag="s1")
                    off = 0
                    for k2, w in widths:
                        kA = slice(k2 * 128, (k2 + 1) * 128)
                        qA = slice(q0 + PASS - w, q0 + PASS)
                        nc.tensor.matmul(sp0[:, off:off + w],
                                         kt[b][0:HD, kA], qt[b][0:HD, qA],
                                         start=True, stop=True)
                        nc.tensor.matmul(sp1[:, off:off + w],
                                         kt[b][HD:128, kA], qt[b][HD:128, qA],
                                         start=True, stop=True)
                        off += w
                    if p >= pump_from:
                        filler.pump(1)
                    # exp (one ACT instruction per head per pair)
                    e0 = epool.tile([128, 1024], bf16, name="e0", tag="e0")
                    e1 = epool.tile([128, 1024], bf16, name="e1", tag="e1")
                    etiles[pi] = (e0, e1, widths)
                    nc.scalar.activation(e0[:, 0:off], sp0[:, 0:off],
                                         AF.Exp, scale=0.125)
                    nc.scalar.activation(e1[:, 0:off], sp1[:, 0:off],
                                         AF.Exp, scale=0.125)
                    # causal mask on diagonal-starting chunks
                    off = 0
                    for k2, w in widths:
                        if 128 * k2 >= q0:
                            nc.vector.tensor_mul(e0[:, off:off + 128],
                                                 e0[:, off:off + 128], diagmask)
                            nc.vector.tensor_mul(e1[:, off:off + 128],
                                                 e1[:, off:off + 128], diagmask)
                        off += w
                    # PV one pair behind (exp of pair pi still in flight)
                    if pi >= 1:
                        do_pv(pi - 1)
                        if p >= pump_from:
                            filler.pump(1)
                do_pv(npair - 1)
                # normalize: rec = 1/denominator, broadcast via K=1 matmul,
                # scale into the staging buffer
                for h, pv in ((0, pv0), (1, pv1)):
                    rec = work.tile([1, PASS], bf16, name="rec", tag="rec")
                    with nc.allow_low_precision(reason="softmax denom to bf16"):
                        nc.vector.reciprocal(rec, pv[HD:HD + 1, :])
                    bc = fil.tile([HD, PASS], f32, name="bc", tag="fil")
                    nc.tensor.matmul(bc, ones_row, rec, start=True, stop=True)
                    # DVE can read only one PSUM operand per instruction:
                    # stage the broadcast through SBUF
                    bcs = work.tile([HD, PASS], bf16, name="bcs", tag="bcs")
                    nc.vector.tensor_copy(bcs, bc)
                    nc.vector.tensor_mul(
                        stage[b][h * HD:(h + 1) * HD, q0:q0 + PASS],
                        pv[0:HD, :], bcs)
                if p >= pump_from:
                    filler.pump(1)
                if p % 2 == 1:
                    issue_a2a(b, p // 2)

        # ---------------- schedule ----------------
        for t in v_proj_thunks(0):
            t()
        # batch 1 projections fill PE gaps during batch 0 attention; gated
        # to pass >= 2 so batch 1's x DMAs have certainly landed
        filler.add(qk_proj_thunks(1))
        filler.add(v_proj_thunks(1))
        attention(0, pump_from=2)
        filler.drain()
        attention(1, pump_from=0)
        filler.drain()

    nc.finalize()
    return nc


def _get_nc():
    with _lock:
        if "nc" not in _cache:
            _cache["nc"] = _build_nc()
        return _cache["nc"]


def _shard_inputs(x, input_ids, Wq, bq, Wk, bk, Wv, bv, Wo, bo):
    import ml_dtypes
    bf16 = ml_dtypes.bfloat16

    x = np.asarray(x, dtype=np.float32)
    xT = np.ascontiguousarray(x.reshape(R, D).T).astype(bf16)
    woT = np.ascontiguousarray(np.asarray(Wo, dtype=np.float32).T).astype(bf16)
    bo_f = np.asarray(bo, dtype=np.float32)
    ids = np.asarray(input_ids).astype(np.int32)
    # ids_r[p, b*NKT + t] = input_ids[b, t*128 + p]
    ids_r = np.ascontiguousarray(ids.reshape(B, NKT, 128).transpose(2, 0, 1)
                                 .reshape(128, B * NKT))
    Wq = np.asarray(Wq, dtype=np.float32)
    Wk = np.asarray(Wk, dtype=np.float32)
    Wv = np.asarray(Wv, dtype=np.float32)
    bq = np.asarray(bq, dtype=np.float32)
    bk = np.asarray(bk, dtype=np.float32)
    bv = np.asarray(bv, dtype=np.float32)

    in_maps = []
    for c in range(NCORES):
        sl = slice(c * LD, (c + 1) * LD)
        in_maps.append({
            "xT": xT,
            "wqT": np.ascontiguousarray(Wq[sl].T).astype(bf16),
            "wkT": np.ascontiguousarray(Wk[sl].T).astype(bf16),
            "wvT": np.ascontiguousarray(Wv[sl].T).astype(bf16),
            "woT": woT,
            "bq": bq[sl].copy(),
            "bk": bk[sl].copy(),
            "bv": bv[sl].copy(),
            "bo": bo_f,
            "ids": ids_r,
        })
    return in_maps


def run(trace=False, **inputs):
    """Run the kernel; returns (output, BassKernelResults)."""
    from concourse.bass_utils import run_bass_kernel_spmd

    nc = _get_nc()
    in_maps = _shard_inputs(**inputs)
    res = run_bass_kernel_spmd(nc, in_maps, core_ids=list(range(NCORES)),
                               trace=trace)
    full = np.empty((B, S, D), dtype=np.float32)
    for c in range(NCORES):
        o = np.asarray(res.results[c]["out"], dtype=np.float32)
        for b in range(B):
            for t in range(2):
                full[b, t * 1024 + c * 128:t * 1024 + (c + 1) * 128, :] = \
                    o[b * RB + t * 128:b * RB + (t + 1) * 128, :]
    return full, res


def kernel(**inputs) -> np.ndarray:
    full, _ = run(trace=False, **inputs)
    return full
